# revision 1
# baseline (speedup 1.0000x reference)
r"""Circulant layer kernel for Trainium2 (8 NeuronCores) — v2.

Math: reference computes mv1 + mv2 = 2 * circconv(d, b) with
d = des @ K, b = body @ K.  Realized via a real-input (half-spectrum)
DFT: only frequencies f = 0..512 are computed (conjugate symmetry),
weights folded into the inverse matrix.

Sharding: 513 frequencies over 8 cores.  Cores 0..7 own f = 64c..64c+63;
the Nyquist f=512 rides in core 0's slot-0 imaginary column (sin(0)=0 is
dead), with a generalized 3-product inverse (G3) that keeps the SPMD
program uniform:
  m0 = Dr*Br, m1 = Di*Bi, mC = Dr*Bi + Di*Br  (per slot t)
  out += m0 @ A + m1 @ B + mC @ C             (A/B/C rows are host consts)
For a normal slot (freq f, w=4/N): A = w cos, B = -w cos, C = -w sin.
For core-0 slot 0: m0 = D0*B0 (A = 2/N), m1 = D512*B512 (B = 2/N*(-1)^j),
C = 0.

Per-core pipeline (all matmul operands bf16, PSUM f32):
  S1  KC^T[s,k]   = sum_j CC[j,s]^T-stationary x KT[j,k]-moving (8 mm, ap 1024)
  T1  KC chunks   = PE-transpose of KC^T                        (8 transposes)
  S2  DB[s,2B]    = sum_k KC[k,s]-stationary x [desT|bodyT]     (8 mm, ap 256)
  T2  dT,bT[b,s]  = PE-transpose of DB halves
  PW  ptA=[m0|m1], ptC=mC  on VectorE
  T3  PA,PC       = PE-transpose of ptA, ptC
  S4  out[b,j]    = PA^T @ G3a + PC^T @ G3b                     (4 mm, ap 512)
Store is bf16 packed in f32 words; host sums the 8 partials.
"""

import numpy as np
import ml_dtypes

import concourse.bass as bass
import concourse.mybir as mybir
import concourse.tile as tile
from concourse.bass_utils import run_bass_kernel_spmd
from concourse.tile_rust import add_dep_helper

B = 128        # batch
D_IN = 1024    # input feature dim (contraction k)
N = 1024       # output feature dim (conv length j)
N_CORES = 8
FPC = 64       # complex frequency slots per core
S = 2 * FPC    # 128 freq columns per core: [0:64]=re(cos), [64:128]=im(-sin)

F32 = mybir.dt.float32
BF16 = mybir.dt.bfloat16

KC_CH = 8      # j-chunks in stage 1 / k-chunks in stage 2
N_KT_DMA = 4   # kt split into this many DMAs (2 j-chunks each)

LAST_RESULT = None
_nc_cache = {}


def _build_nc(warm_iters=11):
    nc = bass.Bass(target_bir_lowering=True)

    # All wire tensors are bf16 packed two-per-f32-word (DMA is element-rate
    # bound).  Per-partition packing: row p holds everything partition p gets.
    # Every instruction may carry ONE semaphore wait, so tensors consumed
    # together share a DMA (cc rides with kt) or are re-staged through DVE.
    CW = N + S                    # bf16 words per (kt chunk | cc chunk) pair
    kts = [nc.declare_dram_parameter(
        f"kt{q}", [128, (2 * CW + (128 if q == 0 else 0)) // 2], F32, False)
        for q in range(N_KT_DMA)]
    dbg = nc.declare_dram_parameter("dbg", [128, (KC_CH * 2 * B + N) // 2], F32, False)
    g3b = nc.declare_dram_parameter("g3b", [128, N // 2], F32, False)
    out = nc.declare_dram_parameter("out", [B, N // 2], F32, isOutput=True)

    with tile.TileContext(nc) as tc:
        with (
            tc.tile_pool(name="main", bufs=1) as pool,
            tc.tile_pool(name="psum", bufs=1, space="PSUM") as pp,
        ):
            # ---- inputs -> SBUF (one serial SP chain; 7 input DMAs) ----
            in_dmas = []
            kt_sb = [pool.tile([128, CW + (64 if q == 0 else 0)], F32,
                               tag=f"kt{q}", name=f"kt{q}")
                     for q in range(N_KT_DMA)]
            for q in range(N_KT_DMA):
                in_dmas.append(nc.sync.dma_start(kt_sb[q][:], kts[q][:, :]))
            dbg_sb = pool.tile([128, (KC_CH * 2 * B + N) // 2], F32, tag="dbg", name="dbg")
            in_dmas.append(nc.scalar.dma_start(dbg_sb[:], dbg[:, :]))
            g3b_sb = pool.tile([128, N // 2], F32, tag="g3b", name="g3b")
            in_dmas.append(nc.scalar.dma_start(g3b_sb[:], g3b[:, :]))

            kt_v = [t.bitcast(BF16) for t in kt_sb]
            koff = lambda c: (128 if c // 2 == 0 else 0) + (c % 2) * CW
            ktc = [kt_v[c // 2][:, koff(c):koff(c) + N] for c in range(KC_CH)]
            cc_sb = [kt_v[c // 2][:, koff(c) + N:koff(c) + CW]
                     for c in range(KC_CH)]
            dbg_v = dbg_sb.bitcast(BF16)          # [128, 2048 + 1024]
            g3b_v = g3b_sb.bitcast(BF16)          # [128, 1024]

            # Staging: operands later consumed alongside engine-produced
            # tiles must themselves be produced by that engine (instructions
            # encode ONE semaphore wait).  Two identity copies: one per
            # consumer engine family.
            id_sb = pool.tile([128, 128], BF16, tag="id2", name="id2")
            nc.vector.tensor_copy(id_sb[:], kt_v[0][:, 0:128])
            dbt_st = pool.tile([128, KC_CH, 2 * B], BF16, tag="dbtst", name="dbtst")
            nc.vector.tensor_copy(
                dbt_st[:], dbg_v[:, :KC_CH * 2 * B].rearrange(
                    "p (c w) -> p c w", c=KC_CH))
            dbtc = [dbt_st[:, c, :] for c in range(KC_CH)]
            g3a_st = pool.tile([128, N], BF16, tag="g3ast", name="g3ast")
            nc.vector.tensor_copy(g3a_st[:], dbg_v[:, KC_CH * 2 * B:])
            g3b_st = pool.tile([128, N], BF16, tag="g3bst", name="g3bst")
            nc.vector.tensor_copy(g3b_st[:], g3b_v[:])
            g3a_sb = g3a_st
            g3b_v2 = g3b_st

            # ---- PE warmup (keeps the clock at 2.4 GHz while DMAs land).
            # Junk matmuls land in ps_out, which S4 later overwrites; ps_out
            # is ultimately read, so the warmup is not dead code.
            wz = pool.tile([128, 640], BF16, tag="wz", name="wz")
            memset_h = nc.gpsimd.memset(wz[:], 0.0)
            ps_out_lo = pp.tile([128, 512], F32, tag="psoutl", name="psoutl")
            ps_out_hi = pp.tile([128, 512], F32, tag="psouth", name="psouth")
            wps = ps_out_lo[:]
            for w in range(warm_iters):
                nc.tensor.matmul(wps, wz[:, :128], wz[:, 128:640],
                                 start=True, stop=True)

            # ---- S1: KC^T[s, k] accumulated over j-chunks ----
            # PSUM is 8 banks x 2KB: big [128, 1024] f32 tile shared by S1
            # and S4 (tag "pskc"), one bf16 bank for all transpose outputs.
            trall = pp.tile([128, 8, 128], BF16, tag="trall", name="trall")
            trall2 = pp.tile([128, 8, 128], BF16, tag="trall2", name="trall2")
            trall3 = pp.tile([128, 8, 128], BF16, tag="trall3", name="trall3")
            ps_kc = [pp.tile([128, 512], F32, tag=f"pskc{h}", name=f"pskc{h}")
                     for h in range(2)]
            for c in range(KC_CH):
                for h in range(2):  # matmul out must stay within a PSUM bank
                    nc.tensor.matmul(ps_kc[h][:],
                                     cc_sb[c], ktc[c][:, h * 512:(h + 1) * 512],
                                     start=(c == 0), stop=(c == KC_CH - 1))
            # big PSUM->SBUF casts are column-rate bound (~1ns/col):
            # split halves across DVE and the Activation engine.
            kcT_lo = pool.tile([128, 512], BF16, tag="kcTl", name="kcTl")
            kcT_hi = pool.tile([128, 512], BF16, tag="kcTh", name="kcTh")
            nc.scalar.copy(kcT_hi[:], ps_kc[1][:])
            nc.vector.tensor_copy(kcT_lo[:], ps_kc[0][:])

            # ---- T1: transpose KC^T chunks -> KC[k, s] chunks ----
            # All 8 into distinct trall slots, then ONE copy (a per-chunk
            # copy after each transpose needs 2 sync waits -> walrus ICE).
            kc_sb = pool.tile([128, KC_CH, S], BF16, tag="kc", name="kc")
            for c in range(KC_CH):
                half = kcT_lo if c < 4 else kcT_hi
                nc.tensor.transpose(trall[:, c, :],
                                    half[:, (c % 4) * 128:(c % 4 + 1) * 128],
                                    id_sb)
            nc.vector.tensor_copy(kc_sb[:], trall[:])
            # keep the PE clock hot while the kc copy runs on DVE
            for w in range(6):
                nc.tensor.matmul(wps[:, 0:256], wz[:, :128], wz[:, 128:384],
                                 start=True, stop=True)

            # ---- S2: DB[s, 2B] = KC^T(stationary KC chunks) @ [desT|bodyT] ----
            ps_db = pp.tile([128, 2 * B], F32, tag="psdb", name="psdb")
            for c in range(KC_CH):
                nc.tensor.matmul(ps_db[:], kc_sb[:, c, :], dbtc[c],
                                 start=(c == 0), stop=(c == KC_CH - 1))
            db_sb = pool.tile([128, 2 * B], BF16, tag="db", name="db")
            nc.vector.tensor_copy(db_sb[:], ps_db[:])

            # ---- T2: dT[b, s], bT[b, s] ----
            ps_dt = trall2[:, 0, :]
            ps_bt = trall2[:, 1, :]
            nc.tensor.transpose(ps_dt, db_sb[:, 0:B], id_sb)
            nc.tensor.transpose(ps_bt, db_sb[:, B:2 * B], id_sb)
            for w in range(6):  # cover the btsw/dt copy window on DVE
                nc.tensor.matmul(wps[:, 0:256], wz[:, :128], wz[:, 128:384],
                                 start=True, stop=True)

            # ---- PW: ptA = [Dr*Br | Di*Bi], ptC2 = [Dr*Bi | Di*Br] ----
            # (DVE reads at most one PSUM operand: stage dT/swapped bT first)
            # DVE reads of trall2 ordered latest-PE-dependency-first so the
            # chain implies the earlier transposes (one-wait encoding).
            dt_sb = pool.tile([128, 128], BF16, tag="dtsb", name="dtsb")
            btsw = pool.tile([128, 128], BF16, tag="btsw", name="btsw")
            nc.vector.tensor_copy(btsw[:, 0:64], ps_bt[:, 64:128])
            nc.vector.tensor_copy(btsw[:, 64:128], ps_bt[:, 0:64])
            nc.vector.tensor_copy(dt_sb[:], ps_dt)
            ptA = pool.tile([128, 128], BF16, tag="ptA", name="ptA")
            ptC2 = pool.tile([128, 128], BF16, tag="ptC2", name="ptC2")
            nc.vector.tensor_mul(ptA[:], dt_sb[:], ps_bt)
            nc.vector.tensor_mul(ptC2[:], dt_sb[:], btsw[:])

            # ---- T3: PA[s, b], PC[s2, b] (G3b rows are duplicated C;C) ----
            ps_pa = trall3[:, 0, :]
            ps_pc = trall3[:, 1, :]
            nc.tensor.transpose(ps_pa, ptA[:], id_sb)
            nc.tensor.transpose(ps_pc, ptC2[:], id_sb)
            # filler: PE idles ~1.5us here waiting on DVE; hold the clock
            for w in range(10):
                nc.tensor.matmul(wps[:, 0:256], wz[:, :128], wz[:, 128:384],
                                 start=True, stop=True)
            pa_sb = pool.tile([128, 128], BF16, tag="pa", name="pa")
            pc_sb = pool.tile([128, 128], BF16, tag="pc", name="pc")
            nc.vector.tensor_copy(pc_sb[:], ps_pc)   # later transpose first:
            nc.vector.tensor_copy(pa_sb[:], ps_pa)   # DVE chain implies pa

            # ---- S4: out[b, j] = PA^T @ G3a + PC^T @ G3b, per 512-half ----
            # Each half is cast and stored as soon as it completes: the store
            # DMA has ~5us proxy latency, so the first trigger must go early.
            out_lo = pool.tile([128, 512], BF16, tag="outlo", name="outlo")
            out_hi = pool.tile([128, 512], BF16, tag="outhi", name="outhi")
            stores = []
            last_mm = cp_lo = cp_hi = None
            for h in range(2):
                pso = ps_out_lo if h == 0 else ps_out_hi
                nc.tensor.matmul(pso[:],
                                 pa_sb[:], g3a_sb[:, h * 512:(h + 1) * 512],
                                 start=True, stop=False)
                last_mm = nc.tensor.matmul(
                    pso[:],
                    pc_sb[:], g3b_v2[:, h * 512:(h + 1) * 512],
                    start=False, stop=True)
                if h == 0:
                    cp_lo = nc.scalar.copy(out_lo[:], pso[:])
                    stores.append(nc.sync.dma_start(
                        out[:, :256], out_lo.bitcast(F32)[:, :]))
                else:
                    cp_hi = nc.vector.tensor_copy(out_hi[:], pso[:])
                    stores.append(nc.scalar.dma_start(
                        out[:, 256:], out_hi.bitcast(F32)[:, :]))

            # ---- tail: absorb every outstanding tick into SP's clock ----
            prev = None
            for dep in [*in_dmas, memset_h, *stores, last_mm, cp_lo, cp_hi]:
                dr = nc.sync.drain(fusable=False)
                add_dep_helper(dr.ins, dep.ins, sync=True,
                               reason="tail: absorb tick into SP clock")
                if prev is not None:
                    add_dep_helper(dr.ins, prev.ins, sync=False,
                                   reason="tail: keep drain chain ordered")
                prev = dr

    return nc


def _bf16_pack(a):
    """float32 (P, W) -> bf16 packed two-per-word as float32 (P, W//2)."""
    bf = np.ascontiguousarray(np.asarray(a, np.float32).astype(ml_dtypes.bfloat16))
    return bf.view(np.uint8).reshape(bf.shape[0], -1).view(np.float32)


def _partition_pack(a):
    """(n*128, W) -> (128, n*W): row p = concat of chunk rows p."""
    r, w = a.shape
    n = r // 128
    return np.ascontiguousarray(
        a.reshape(n, 128, w).transpose(1, 0, 2).reshape(128, n * w))


def _constants():
    """Per-core CC [N, S], G3a [128, N], G3b [64, N] float32."""
    j = np.arange(N, dtype=np.float64)
    alt = np.cos(np.pi * j)                     # (-1)^j
    ccs, g3as, g3bs = [], [], []
    for c in range(N_CORES):
        f = np.arange(c * FPC, (c + 1) * FPC, dtype=np.float64)
        ang = 2.0 * np.pi * np.outer(j, f) / N             # (j, t)
        cc_re = np.cos(ang)
        cc_im = -np.sin(ang)
        angT = ang.T                                        # (t, j)
        w = 4.0 / N
        A = w * np.cos(angT)                                # m0 rows
        Bm = -w * np.cos(angT)                              # m1 rows
        C = -w * np.sin(angT)                               # mC rows
        if c == 0:
            cc_im[:, 0] = alt                               # f=512 cos column
            A[0, :] = 2.0 / N                               # m0 = D0*B0
            Bm[0, :] = (2.0 / N) * alt                      # m1 = D512*B512
            C[0, :] = 0.0
        cc = np.concatenate([cc_re, cc_im], axis=1)         # (N, 128)
        ccs.append(np.ascontiguousarray(cc, np.float32))
        g3as.append(np.ascontiguousarray(
            np.concatenate([A, Bm], axis=0), np.float32))       # (128, N)
        g3bs.append(np.ascontiguousarray(
            np.concatenate([C, C], axis=0), np.float32))        # (128, N)
    return ccs, g3as, g3bs


def kernel(des, body, kernel):
    global LAST_RESULT
    K = np.asarray(kernel, dtype=np.float32)
    des = np.asarray(des, dtype=np.float32)
    body = np.asarray(body, dtype=np.float32)

    kt_bf = _partition_pack(
        K.T.astype(ml_dtypes.bfloat16).astype(np.float32))  # (128, 8*1024) f32
    dbt = np.concatenate([des.T, body.T], axis=1)       # (1024, 256)
    dbt_pk = _partition_pack(_bf16_pack(dbt))           # (128, 8*128) words
    id_pk = _bf16_pack(np.eye(128, dtype=np.float32))   # (128, 64) words

    ccs, g3as, g3bs = _constants()
    ktqs, dbgs, g3bs_pk = [], [], []
    for c in range(N_CORES):
        cc_bf = _partition_pack(ccs[c])                 # (128, 8*128) f32
        # interleave per chunk: [kt_c (1024) | cc_c (128)] then bf16-pack
        ktcc = np.concatenate(
            [kt_bf.reshape(128, KC_CH, N), cc_bf.reshape(128, KC_CH, S)],
            axis=2).reshape(128, KC_CH * (N + S))
        ktcc_pk = _bf16_pack(ktcc)                      # (128, 8*(1024+128)/2)
        w = ktcc_pk.shape[1] // N_KT_DMA
        qs = [ktcc_pk[:, q * w:(q + 1) * w] for q in range(N_KT_DMA)]
        qs[0] = np.concatenate([id_pk, qs[0]], axis=1)  # identity rides kt0
        ktqs.append([np.ascontiguousarray(a) for a in qs])
        dbgs.append(np.ascontiguousarray(
            np.concatenate([dbt_pk, _bf16_pack(g3as[c])], axis=1)))
        g3bs_pk.append(np.ascontiguousarray(_bf16_pack(g3bs[c])))

    if "nc" not in _nc_cache:
        _nc_cache["nc"] = _build_nc()
    nc = _nc_cache["nc"]

    in_maps = [
        {**{f"kt{q}": ktqs[c][q] for q in range(N_KT_DMA)},
         "dbg": dbgs[c],
         "g3b": g3bs_pk[c]}
        for c in range(N_CORES)
    ]
    res = run_bass_kernel_spmd(nc, in_maps, list(range(N_CORES)))
    LAST_RESULT = res
    out = np.zeros((B, N), dtype=np.float32)
    for r in res.results:
        w = np.ascontiguousarray(np.asarray(r["out"], np.float32))
        bf = w.view(np.uint8).reshape(B, -1).view(ml_dtypes.bfloat16)
        out += bf.astype(np.float32)
    return out



# revision 8
# speedup vs baseline: 1.1569x; 1.1569x over previous
r"""Circulant layer kernel for Trainium2 (8 NeuronCores) — v3.

Math (same as v2): reference computes mv1 + mv2 = 2 * circconv(d, b)
with d = des @ K, b = body @ K.  Real-input half-spectrum DFT: cores
0..7 own freqs f = 64c..64c+63; Nyquist f=512 rides core 0's slot-0
imaginary column with the generalized 3-product inverse (G3).

v3 structural changes vs v2 (40.9us -> target ~34us):
  * K^T streams on BOTH hardware DMA queues (SP + ACT), split by
    j-chunk pairs and k-halves; cc/dbt ride the gpsimd SWDGE queue.
    (v2 put all of kt on one queue at ~190GB/s — the single-queue
    stream, not PE, set the critical path.)
  * k-half phasing: S1 (KC^T = CC^T K^T) accumulates k-half 0 in PSUM
    bank 0 and k-half 1 in bank 1, so T1/S2 for half 0 run while
    half 1 is still streaming in.
  * The pointwise spectral products are computed directly in [s, b]
    layout from S2's output (DVE ops with partition-base-shifted
    operands — verified on HW), eliminating T2, T3 and their staging
    copies entirely:
      ptA[p, b]        = db[p, b] * db[p, B+b]          (p = 0..127)
      ptC2[p, b]       = db[p, b] * db[(p+64)%128, B+b] (two half ops)
    ptA/ptC2 feed S4 as stationaries with G3a / duplicated-C moving.
  * S4 + cast + store issue per 512-col PSUM bank as soon as ready.

Fixed costs measured by probe: ~8.3us preamble, ~2us DMA issue->land,
~2us store issue->tick, ~8.3us after last store tick.
"""

import numpy as np
import ml_dtypes

import concourse.bass as bass
import concourse.mybir as mybir
import concourse.tile as tile
from concourse.bass_utils import run_bass_kernel_spmd
from concourse.tile_rust import add_dep_helper

B = 128        # batch
D_IN = 1024    # input feature dim (contraction k)
N = 1024       # output feature dim (conv length j)
N_CORES = 8
FPC = 64       # complex frequency slots per core
S = 2 * FPC    # 128 freq columns per core: [0:64]=re(cos), [64:128]=im(-sin)

F32 = mybir.dt.float32
BF16 = mybir.dt.bfloat16

LAST_RESULT = None
_nc_cache = {}


def _build_nc():
    nc = bass.Bass(target_bir_lowering=True)

    # --- DRAM params (bf16 packed two-per-f32-word) ---
    # SP queue: [id | kt h0 c0c1] [kt h0 c2c3] [kt h1 c0c1] [kt h1 c2c3] [g3a]
    # ACT queue: [kt h0 c4c5] [kt h0 c6c7] [kt h1 c4c5] [kt h1 c6c7] [g3b2]
    # GP queue: [cc] [dbt]
    # each kt (c,h) block: [128, 512] bf16 = 256 f32 words; pairs = 512 words
    sp1 = nc.declare_dram_parameter("sp1", [128, 64 + 1024], F32, False)
    sp2 = nc.declare_dram_parameter("sp2", [128, 1536], F32, False)
    ac1 = nc.declare_dram_parameter("ac1", [128, 1024], F32, False)
    ac2 = nc.declare_dram_parameter("ac2", [128, 1536], F32, False)
    cc = nc.declare_dram_parameter("cc", [128, 512], F32, False)
    dbt = nc.declare_dram_parameter("dbt", [128, 1024], F32, False)
    out = nc.declare_dram_parameter("out", [B, N // 2], F32, isOutput=True)

    with tile.TileContext(nc) as tc:
        with (
            tc.tile_pool(name="main", bufs=1) as pool,
            tc.tile_pool(name="psum", bufs=1, space="PSUM") as pp,
        ):
            # ---- input DMAs, phase-ordered per queue ----
            sp1_sb = pool.tile([128, 64 + 1024], F32, tag="sp1", name="sp1")
            sp2_sb = pool.tile([128, 1536], F32, tag="sp2", name="sp2")
            ac1_sb = pool.tile([128, 1024], F32, tag="ac1", name="ac1")
            ac2_sb = pool.tile([128, 1536], F32, tag="ac2", name="ac2")
            cc_sb = pool.tile([128, 512], F32, tag="cc", name="cc")
            dbt_sb = pool.tile([128, 1024], F32, tag="dbt", name="dbt")

            in_dmas = []
            in_dmas.append(nc.sync.dma_start(sp1_sb[:], sp1[:, :]))
            in_dmas.append(nc.sync.dma_start(sp2_sb[:], sp2[:, :]))
            in_dmas.append(nc.scalar.dma_start(ac1_sb[:], ac1[:, :]))
            in_dmas.append(nc.scalar.dma_start(ac2_sb[:], ac2[:, :]))
            in_dmas.append(nc.gpsimd.dma_start(cc_sb[:], cc[:, :]))
            in_dmas.append(nc.gpsimd.dma_start(dbt_sb[:], dbt[:, :]))

            # bf16 views
            id_v = sp1_sb.bitcast(BF16)[:, 0:128]
            # kt[c][h] -> [128, 512] bf16 view
            sp1v = sp1_sb.bitcast(BF16)
            sp2v = sp2_sb.bitcast(BF16)
            ac1v = ac1_sb.bitcast(BF16)
            ac2v = ac2_sb.bitcast(BF16)
            ktv = {}
            for c in range(4):
                ktv[(c, 0)] = sp1v[:, 128 + c * 512:128 + (c + 1) * 512]
                ktv[(c, 1)] = sp2v[:, c * 512:(c + 1) * 512]
                ktv[(4 + c, 0)] = ac1v[:, c * 512:(c + 1) * 512]
                ktv[(4 + c, 1)] = ac2v[:, c * 512:(c + 1) * 512]
            g3a_v = sp2v[:, 2048:3072]            # [128, 1024]
            g3b_v = ac2v[:, 2048:3072]            # [128, 1024]
            cc_v = cc_sb.bitcast(BF16).rearrange(
                "p (c s) -> p c s", c=8)          # [128, 8, 128]
            dbt_v = dbt_sb.bitcast(BF16).rearrange(
                "p (c w) -> p c w", c=8)          # [128, 8, 256]

            # ---- PSUM layout ----
            ps_kc0 = pp.tile([128, 512], F32, tag="pskc0", name="pskc0")
            ps_kc1 = pp.tile([128, 512], F32, tag="pskc1", name="pskc1")
            ps_db = pp.tile([128, 2 * B], F32, tag="psdb", name="psdb")
            trall = pp.tile([128, 4, 128], BF16, tag="trall", name="trall")
            trall2 = pp.tile([128, 4, 128], BF16, tag="trall2", name="trall2")
            ps_out_lo = pp.tile([128, 512], F32, tag="psoutl", name="psoutl")
            ps_out_hi = pp.tile([128, 512], F32, tag="psouth", name="psouth")

            # ---- PE warmup: junk matmuls into ps_out (S4 overwrites) ----
            wz = pool.tile([128, 640], BF16, tag="wz", name="wz")
            memset_h = nc.gpsimd.memset(wz[:], 0.0)
            for w in range(4):
                nc.tensor.matmul(ps_out_lo[:], wz[:, :128], wz[:, 128:640],
                                 start=True, stop=True)

            # ---- S1 phase 0: ps_kc0[s, k0:512] = sum_j cc[j,s]^T kt[j, h0] ----
            # mm order follows expected landing: SP pair (0,1), ACT (4,5),
            # SP (2,3), ACT (6,7)
            h0_order = [0, 1, 4, 5, 2, 3, 6, 7]
            for i, c in enumerate(h0_order):
                nc.tensor.matmul(ps_kc0[:], cc_v[:, c, :], ktv[(c, 0)],
                                 start=(i == 0), stop=(i == 7))

            # ---- T1 phase 0: transpose KC^T[:, 0:512] -> kc chunks 0..3 ----
            kcT0 = pool.tile([128, 512], BF16, tag="kcT0", name="kcT0")
            nc.scalar.copy(kcT0[:], ps_kc0[:])
            for c in range(4):
                nc.tensor.transpose(trall[:, c, :],
                                    kcT0[:, c * 128:(c + 1) * 128], id_v)
            kc_lo = pool.tile([128, 4, 128], BF16, tag="kclo", name="kclo")
            nc.vector.tensor_copy(
                kc_lo[:].rearrange("p c s -> p (c s)"),
                trall[:].rearrange("p c s -> p (c s)"))

            # ---- S1 phase 1 (first half): overlap with S2p0 setup ----
            h1_order = [0, 1, 4, 5, 2, 3, 6, 7]
            for i, c in enumerate(h1_order[:2]):
                nc.tensor.matmul(ps_kc1[:], cc_v[:, c, :], ktv[(c, 1)],
                                 start=(i == 0), stop=False)

            # ---- S2 partial 0: ps_db += kc[k0 chunks] @ dbt ----
            for c in range(4):
                nc.tensor.matmul(ps_db[:], kc_lo[:, c, :], dbt_v[:, c, :],
                                 start=(c == 0), stop=False)

            # ---- S1 phase 1 (rest) ----
            for i, c in enumerate(h1_order[2:]):
                nc.tensor.matmul(ps_kc1[:], cc_v[:, c, :], ktv[(c, 1)],
                                 start=False, stop=(i == 5))

            # ---- T1 phase 1 ----
            kcT1 = pool.tile([128, 512], BF16, tag="kcT1", name="kcT1")
            nc.scalar.copy(kcT1[:], ps_kc1[:])
            for c in range(4):
                nc.tensor.transpose(trall2[:, c, :],
                                    kcT1[:, c * 128:(c + 1) * 128], id_v)
            # separate PSUM tile (trall2): a second read of the same PSUM
            # tile needs two sync waits, which compute instrs can't encode
            kc_hi = pool.tile([128, 4, 128], BF16, tag="kchi", name="kchi")
            nc.vector.tensor_copy(
                kc_hi[:].rearrange("p c s -> p (c s)"),
                trall2[:].rearrange("p c s -> p (c s)"))

            # ---- S2 partial 1 ----
            for c in range(4):
                nc.tensor.matmul(ps_db[:], kc_hi[:, c, :],
                                 dbt_v[:, 4 + c, :],
                                 start=False, stop=(c == 3))

            # ---- PW in [s, b] layout (no transposes) ----
            db_sb = pool.tile([128, 2 * B], BF16, tag="db", name="db")
            nc.vector.tensor_copy(db_sb[:], ps_db[:])
            # partition-swapped copy of the B half (tensor_tensor requires
            # same start partition on all APs; tensor_copy does not)
            dbsw = pool.tile([128, B], BF16, tag="dbsw", name="dbsw")
            nc.vector.tensor_copy(dbsw[0:64, :], db_sb[64:128, B:2 * B])
            nc.vector.tensor_copy(dbsw[64:128, :], db_sb[0:64, B:2 * B])
            ptA = pool.tile([128, B], BF16, tag="ptA", name="ptA")
            ptC2 = pool.tile([128, B], BF16, tag="ptC2", name="ptC2")
            nc.vector.tensor_mul(ptA[:], db_sb[:, 0:B], db_sb[:, B:2 * B])
            nc.vector.tensor_mul(ptC2[:], db_sb[:, 0:B], dbsw[:])

            # ---- S4 per bank: out[b, j] = ptA^T G3a + ptC2^T G3b2 ----
            out_lo = pool.tile([128, 512], BF16, tag="outlo", name="outlo")
            out_hi = pool.tile([128, 512], BF16, tag="outhi", name="outhi")
            stores = []
            nc.tensor.matmul(ps_out_lo[:], ptA[:], g3a_v[:, 0:512],
                             start=True, stop=False)
            nc.tensor.matmul(ps_out_lo[:], ptC2[:], g3b_v[:, 0:512],
                             start=False, stop=True)
            cp_lo = nc.scalar.copy(out_lo[:], ps_out_lo[:])
            stores.append(nc.sync.dma_start(out[:, :256],
                                            out_lo.bitcast(F32)[:, :]))
            nc.tensor.matmul(ps_out_hi[:], ptA[:], g3a_v[:, 512:1024],
                             start=True, stop=False)
            last_mm = nc.tensor.matmul(ps_out_hi[:], ptC2[:],
                                       g3b_v[:, 512:1024],
                                       start=False, stop=True)
            cp_hi = nc.vector.tensor_copy(out_hi[:], ps_out_hi[:])
            stores.append(nc.scalar.dma_start(out[:, 256:],
                                              out_hi.bitcast(F32)[:, :]))

            # ---- tail: absorb every outstanding tick into SP's clock ----
            prev = None
            for dep in [*in_dmas, memset_h, *stores, last_mm, cp_lo, cp_hi]:
                dr = nc.sync.drain(fusable=False)
                add_dep_helper(dr.ins, dep.ins, sync=True,
                               reason="tail: absorb tick into SP clock")
                if prev is not None:
                    add_dep_helper(dr.ins, prev.ins, sync=False,
                                   reason="tail: keep drain chain ordered")
                prev = dr

    return nc


def _bf16_pack(a):
    """float32 (P, W) -> bf16 packed two-per-word as float32 (P, W//2)."""
    bf = np.ascontiguousarray(np.asarray(a, np.float32).astype(ml_dtypes.bfloat16))
    return bf.view(np.uint8).reshape(bf.shape[0], -1).view(np.float32)


def _partition_pack(a):
    """(n*128, W) -> (128, n*W): row p = concat of chunk rows p."""
    r, w = a.shape
    n = r // 128
    return np.ascontiguousarray(
        a.reshape(n, 128, w).transpose(1, 0, 2).reshape(128, n * w))


def _constants():
    """Per-core CC [N, S], G3a [128, N], G3b2 [128, N] float32."""
    j = np.arange(N, dtype=np.float64)
    alt = np.cos(np.pi * j)                     # (-1)^j
    ccs, g3as, g3bs = [], [], []
    for c in range(N_CORES):
        f = np.arange(c * FPC, (c + 1) * FPC, dtype=np.float64)
        ang = 2.0 * np.pi * np.outer(j, f) / N             # (j, t)
        cc_re = np.cos(ang)
        cc_im = -np.sin(ang)
        angT = ang.T                                        # (t, j)
        w = 4.0 / N
        A = w * np.cos(angT)                                # m0 rows
        Bm = -w * np.cos(angT)                              # m1 rows
        C = -w * np.sin(angT)                               # mC rows
        if c == 0:
            cc_im[:, 0] = alt                               # f=512 cos column
            A[0, :] = 2.0 / N                               # m0 = D0*B0
            Bm[0, :] = (2.0 / N) * alt                      # m1 = D512*B512
            C[0, :] = 0.0
        cc_full = np.concatenate([cc_re, cc_im], axis=1)    # (N, 128)
        ccs.append(np.ascontiguousarray(cc_full, np.float32))
        g3as.append(np.ascontiguousarray(
            np.concatenate([A, Bm], axis=0), np.float32))       # (128, N)
        g3bs.append(np.ascontiguousarray(
            np.concatenate([C, C], axis=0), np.float32))        # (128, N)
    return ccs, g3as, g3bs


def kernel(des, body, kernel):
    global LAST_RESULT
    K = np.asarray(kernel, dtype=np.float32)
    des = np.asarray(des, dtype=np.float32)
    body = np.asarray(body, dtype=np.float32)

    # K^T as bf16 blocks: block (c, h) = K^T[c*128:(c+1)*128, h*512:(h+1)*512]
    ktb = K.T.astype(ml_dtypes.bfloat16)                # (1024 j, 1024 k)
    def ktpk(c, h):
        blk = np.ascontiguousarray(
            ktb[c * 128:(c + 1) * 128, h * 512:(h + 1) * 512], np.float32)
        return _bf16_pack(blk)                          # (128, 256) words

    id_pk = _bf16_pack(np.eye(128, dtype=np.float32))   # (128, 64) words
    dbt_np = np.concatenate([des.T, body.T], axis=1)    # (1024, 256)
    dbt_pk = _partition_pack(_bf16_pack(dbt_np))        # (128, 1024) words

    ccs, g3as, g3bs = _constants()
    in_maps = []
    for c in range(N_CORES):
        cc_pk = _partition_pack(_bf16_pack(ccs[c]))     # (128, 512) words
        m = {
            "sp1": np.ascontiguousarray(np.concatenate(
                [id_pk, ktpk(0, 0), ktpk(1, 0), ktpk(2, 0), ktpk(3, 0)],
                axis=1)),
            "sp2": np.ascontiguousarray(np.concatenate(
                [ktpk(0, 1), ktpk(1, 1), ktpk(2, 1), ktpk(3, 1),
                 _bf16_pack(g3as[c])], axis=1)),
            "ac1": np.ascontiguousarray(np.concatenate(
                [ktpk(4, 0), ktpk(5, 0), ktpk(6, 0), ktpk(7, 0)], axis=1)),
            "ac2": np.ascontiguousarray(np.concatenate(
                [ktpk(4, 1), ktpk(5, 1), ktpk(6, 1), ktpk(7, 1),
                 _bf16_pack(g3bs[c])], axis=1)),
            "cc": cc_pk,
            "dbt": dbt_pk,
        }
        in_maps.append(m)

    if "nc" not in _nc_cache:
        _nc_cache["nc"] = _build_nc()
    nc = _nc_cache["nc"]

    res = run_bass_kernel_spmd(nc, in_maps, list(range(N_CORES)))
    LAST_RESULT = res
    out = np.zeros((B, N), dtype=np.float32)
    for r in res.results:
        w = np.ascontiguousarray(np.asarray(r["out"], np.float32))
        bf = w.view(np.uint8).reshape(B, -1).view(ml_dtypes.bfloat16)
        out += bf.astype(np.float32)
    return out
